# revision 29
# baseline (speedup 1.0000x reference)
"""Trainium2 Bass kernel for single-head causal attention (B=4, S=2048, D=1024).

Sharding: 8 cores = 4 batches x 2 query-halves (1024 q-rows each). Every core
runs the identical program on full-length keys (2048); causality enters via a
per-core additive mask input, so one SPMD NEFF serves all cores.

Per-core dataflow (all f32 storage, float32r matmuls, fp32 PSUM accumulate):
  A1: kT[do,k]  = Wk^T @ xkT      (kT resident in SBUF, 64KB/part)
  A2: v[s,dv]   = xv @ Wv         (spilled to DRAM, re-streamed in B2)
  A3: qT[do,q]  = Wq^T @ xqT      (resident, 32KB/part)
  B1: S[q,k]    = qT^T kT + mask; softmax rows via 2x ACT-exp passes
      PT        = P^T per 128x128 PE transpose
  B2: OT[dv,q]  = sum_k v[k,dv] * PT[k,q]   (PSUM accum over k-chunks)
  C : out[q,d]  = OT^T @ Wo + bo
"""

import os
import sys

import numpy as np

for _p in ("/opt/trn_rl_repo",):
    if os.path.isdir(_p) and _p not in sys.path:
        sys.path.insert(0, _p)

B, S, D = 4, 2048, 1024
P = 128
DK = D // P          # 8 contraction chunks of 128
QROWS = S // 2       # 1024 q rows per core
NJOB = 2             # jobs per core (512 q rows each)
QH = QROWS // NJOB   # 512
KB = S // 512        # 4 key blocks of 512
KC = S // P          # 16 key chunks of 128
NEG = -1e9
ISCALE = 1.0 / np.sqrt(np.float32(D))

_CACHE = {}


def _build_nc():
    from contextlib import ExitStack

    import concourse.bass as bass
    import concourse.mybir as mybir
    import concourse.tile as tile
    from concourse import bacc
    from concourse.masks import make_identity

    F32 = mybir.dt.float32
    FR = mybir.dt.float32r
    Ident = mybir.ActivationFunctionType.Identity
    Exp = mybir.ActivationFunctionType.Exp
    Ln = mybir.ActivationFunctionType.Ln
    AX = mybir.AxisListType.X
    MAX = mybir.AluOpType.max

    nc = bacc.Bacc(None, target_bir_lowering=False, debug=False)

    xqT = nc.dram_tensor("xqT", [D, QROWS], FR, kind="ExternalInput")
    xkT = nc.dram_tensor("xkT", [D, S], FR, kind="ExternalInput")
    xvT = nc.dram_tensor("xvT", [D, S], FR, kind="ExternalInput")
    Wq = nc.dram_tensor("Wq", [D, D], FR, kind="ExternalInput")
    Wk = nc.dram_tensor("Wk", [D, D], FR, kind="ExternalInput")
    Wv = nc.dram_tensor("Wv", [D, D], FR, kind="ExternalInput")
    Wo = nc.dram_tensor("Wo", [D, D], FR, kind="ExternalInput")
    bqp = nc.dram_tensor("bqp", [P, DK], F32, kind="ExternalInput")
    bkp = nc.dram_tensor("bkp", [P, DK], F32, kind="ExternalInput")
    bvr = nc.dram_tensor("bvr", [1, D], F32, kind="ExternalInput")
    bor = nc.dram_tensor("bor", [1, D], F32, kind="ExternalInput")
    mask = nc.dram_tensor("mask", [QROWS, S], mybir.dt.bfloat16, kind="ExternalInput")
    out = nc.dram_tensor("out", [QROWS, D], F32, kind="ExternalOutput")
    vspill = nc.dram_tensor("vspill", [S, D], FR, kind="Internal")

    wq_r = Wq.rearrange("(po pi) n -> pi po n", pi=P)
    wk_r = Wk.rearrange("(po pi) n -> pi po n", pi=P)
    wv_r = Wv.rearrange("(po pi) n -> pi po n", pi=P)
    wo_r = Wo.rearrange("(po pi) n -> pi po n", pi=P)
    xq_r = xqT.rearrange("(po pi) n -> pi po n", pi=P)
    xk_r = xkT.rearrange("(po pi) n -> pi po n", pi=P)
    xv_r = xvT.rearrange("(po pi) n -> pi po n", pi=P)

    with ExitStack() as ctx:
        tc = ctx.enter_context(tile.TileContext(nc))

        const = ctx.enter_context(tc.tile_pool(name="const", bufs=1))
        wp = ctx.enter_context(tc.tile_pool(name="wp", bufs=1))
        ktp = ctx.enter_context(tc.tile_pool(name="ktp", bufs=1))
        qtp = ctx.enter_context(tc.tile_pool(name="qtp", bufs=1))

        mm_ps = ctx.enter_context(tc.tile_pool(name="mm_ps", bufs=2, space="PSUM"))
        tp_ps = ctx.enter_context(tc.tile_pool(name="tp_ps", bufs=2, space="PSUM"))
        pv_ps = ctx.enter_context(tc.tile_pool(name="pv_ps", bufs=1, space="PSUM"))

        # ---- constants (persistent) ----
        ident = const.tile([P, P], FR)
        bqp_sb = const.tile([P, DK], F32)
        bkp_sb = const.tile([P, DK], F32)
        bvrep = const.tile([P, D], F32, tag="brep")
        linvall = const.tile([P, DK], F32)

        # ---- persistent activations ----
        kT = ktp.tile([P, DK, S], FR)       # 64KB/part
        qT = qtp.tile([P, DK, QROWS], FR)   # 32KB/part

        wsb = wp.tile([P, DK, D], FR, tag="W")

        # ---- Phase A: projections ----
        with tc.tile_pool(name="astream", bufs=2) as ast:
            # A1: kT[do, k] = sum_di Wk[di,do] * xkT[di,k]  (+ bk)
            # issue order matters: DMA issue serializes ~1us each on Sync
            xkts = []
            xkt0 = ast.tile([P, DK, 512], FR, tag="xs", name="xkt0")
            nc.sync.dma_start(xkt0[:], xk_r[:, :, 0:512])
            xkts.append(xkt0)
            nc.sync.dma_start(wsb[:, :, 0:P], wk_r[:, :, 0:P])
            xkt1 = ast.tile([P, DK, 512], FR, tag="xs", name="xkt1")
            nc.sync.dma_start(xkt1[:], xk_r[:, :, 512:1024])
            xkts.append(xkt1)
            nc.sync.dma_start(wsb[:, :, P:512], wk_r[:, :, P:512])
            nc.sync.dma_start(wsb[:, :, 512:D], wk_r[:, :, 512:D])
            for kb in (2, 3):
                xkt = ast.tile([P, DK, 512], FR, tag="xs", name=f"xkt{kb}")
                nc.sync.dma_start(xkt[:], xk_r[:, :, kb * 512 : (kb + 1) * 512])
                xkts.append(xkt)
            # tiny constants go on the gpsimd queue (not behind 16MB of W/x)
            nc.gpsimd.dma_start(bkp_sb[:], bkp[:, :])
            nc.gpsimd.dma_start(bqp_sb[:], bqp[:, :])
            nc.gpsimd.dma_start(bvrep[:], bvr[0, :].partition_broadcast(P))
            ident32 = ast.tile([P, P], F32, tag="id32")
            make_identity(nc, ident32)
            nc.vector.tensor_copy(out=ident[:], in_=ident32[:])
            for kb in range(KB):
                xkt = xkts[kb]
                for do in range(DK):
                    gpool, gtag = ((mm_ps, "mm"), (tp_ps, "tp"))[(kb * DK + do) % 2]
                    ps = gpool.tile([P, 512], F32, tag=gtag)
                    for di in range(DK):
                        nc.tensor.matmul(
                            ps[:],
                            lhsT=wsb[:, di, do * P : (do + 1) * P],
                            rhs=xkt[:, di, :],
                            start=(di == 0),
                            stop=(di == DK - 1),
                        )
                    nc.scalar.activation(
                        out=kT[:, do, kb * 512 : (kb + 1) * 512],
                        in_=ps[:],
                        func=Ident,
                        bias=bkp_sb[:, do : do + 1],
                        scale=1.0,
                    )

            # A2: v[s, dv] = sum_di xvT[di,s] * Wv[di,dv]  (+ bv) -> spill DRAM
            wsb2 = wp.tile([P, DK, D], FR, tag="W")
            for wc in range(DK):
                nc.sync.dma_start(
                    wsb2[:, :, wc * P : (wc + 1) * P],
                    wv_r[:, :, wc * P : (wc + 1) * P],
                )
            for sb in range(4):
                xvt = ast.tile([P, DK, 512], FR, tag="xs", name=f"xvt{sb}")
                nc.sync.dma_start(xvt[:], xv_r[:, :, sb * 512 : (sb + 1) * 512])
                for sl in range(4):
                    sc = sb * 4 + sl
                    vstage = ast.tile([P, D], FR, tag="vstage", name=f"vst{sc}")
                    for dh in range(2):
                        gpool, gtag = ((mm_ps, "mm"), (tp_ps, "tp"))[(sc * 2 + dh) % 2]
                        ps = gpool.tile([P, 512], F32, tag=gtag)
                        for di in range(DK):
                            nc.tensor.matmul(
                                ps[:],
                                lhsT=xvt[:, di, sl * P : (sl + 1) * P],
                                rhs=wsb2[:, di, dh * 512 : (dh + 1) * 512],
                                start=(di == 0),
                                stop=(di == DK - 1),
                            )
                        nc.vector.tensor_add(
                            out=vstage[:, dh * 512 : (dh + 1) * 512],
                            in0=ps[:],
                            in1=bvrep[:, dh * 512 : (dh + 1) * 512],
                        )
                    nc.sync.dma_start(vspill[sc * P : (sc + 1) * P, :], vstage[:])

            # A3: qT[do, q] = sum_di Wq[di,do] * xqT[di,q]  (+ bq)
            wsb3 = wp.tile([P, DK, D], FR, tag="W")
            for wc in range(DK):
                nc.sync.dma_start(
                    wsb3[:, :, wc * P : (wc + 1) * P],
                    wq_r[:, :, wc * P : (wc + 1) * P],
                )
            for j in range(NJOB):
                xqt = ast.tile([P, DK, 512], FR, tag="xs")
                nc.sync.dma_start(xqt[:], xq_r[:, :, j * QH : (j + 1) * QH])
                for do in range(DK):
                    gpool, gtag = ((mm_ps, "mm"), (tp_ps, "tp"))[(j * DK + do) % 2]
                    ps = gpool.tile([P, 512], F32, tag=gtag)
                    for di in range(DK):
                        nc.tensor.matmul(
                            ps[:],
                            lhsT=wsb3[:, di, do * P : (do + 1) * P],
                            rhs=xqt[:, di, :],
                            start=(di == 0),
                            stop=(di == DK - 1),
                        )
                    nc.scalar.activation(
                        out=qT[:, do, j * QH : (j + 1) * QH],
                        in_=ps[:],
                        func=Ident,
                        bias=bqp_sb[:, do : do + 1],
                        scale=1.0,
                    )

        # Wo for phase C (single W slot; load ordered after Wq's last use)
        wsb4 = wp.tile([P, DK, D], FR, tag="W")
        for wc in range(DK):
            nc.sync.dma_start(
                wsb4[:, :, wc * P : (wc + 1) * P],
                wo_r[:, :, wc * P : (wc + 1) * P],
            )
        borep = const.tile([P, D], F32, tag="brep")
        nc.sync.dma_start(borep[:], bor[0, :].partition_broadcast(P))

        # ---- B/C-phase pools (allocated in the space astream released) ----
        ptp = ctx.enter_context(tc.tile_pool(name="ptp", bufs=1))
        otp = ctx.enter_context(tc.tile_pool(name="otp", bufs=1))
        sp = ctx.enter_context(tc.tile_pool(name="sp", bufs=2))
        mp = ctx.enter_context(tc.tile_pool(name="mp", bufs=1))
        vtp = ctx.enter_context(tc.tile_pool(name="vtp", bufs=3))
        outp = ctx.enter_context(tc.tile_pool(name="outp", bufs=1))
        statp = ctx.enter_context(tc.tile_pool(name="statp", bufs=1))

        PT = ptp.tile([P, KC, QH], FR)      # 32KB/part (per job, reused)
        OT = otp.tile([P, DK, QH], FR)      # 16KB/part (per job, reused)

        # ---- Phases B/C, software-pipelined across (job, q-subtile) ----
        # PE queue order: QK(s0) QK(s1) tp(s0) QK(s2) tp(s1) ... so each
        # stage's transposes wait out its softmax under the NEXT stage's QK
        # matmuls, keeping the PE dense (HAM stays at full clock).

        def emit_qk_softmax(j, qi):
            qg = j * 4 + qi
            S_sb = sp.tile([P, S], FR, tag="S", name=f"S_{qg}")
            mx4 = statp.tile([P, KB], F32, tag="mx4", name=f"mx4_{qg}")
            for kb in range(KB):
                mkt = mp.tile([P, 512], mybir.dt.bfloat16, tag="mask", name=f"mkt_{qg}_{kb}")
                nc.sync.dma_start(
                    mkt[:],
                    mask[qg * P : (qg + 1) * P, kb * 512 : (kb + 1) * 512],
                )
                ps = mm_ps.tile([P, 512], F32, tag="mm")
                for di in range(DK):
                    nc.tensor.matmul(
                        ps[:],
                        lhsT=qT[:, di, qg * P : (qg + 1) * P],
                        rhs=kT[:, di, kb * 512 : (kb + 1) * 512],
                        start=(di == 0),
                        stop=(di == DK - 1),
                    )
                nc.vector.tensor_add(
                    out=S_sb[:, kb * 512 : (kb + 1) * 512],
                    in0=ps[:],
                    in1=mkt[:],
                )
                nc.vector.tensor_reduce(
                    out=mx4[:, kb : kb + 1],
                    in_=S_sb[:, kb * 512 : (kb + 1) * 512],
                    axis=AX,
                    op=MAX,
                )
            mraw = statp.tile([P, 1], F32, tag="mraw", name=f"mraw_{qg}")
            nc.vector.tensor_reduce(out=mraw[:], in_=mx4[:], axis=AX, op=MAX)
            nm32 = statp.tile([P, 1], F32, tag="nm32", name=f"nm32_{qg}")
            nc.vector.tensor_scalar_mul(nm32[:], mraw[:], -float(ISCALE))
            # single in-place exp with fused row-sum, then scale by 1/l
            lsum = statp.tile([P, 1], F32, tag="lsum", name=f"lsum_{qg}")
            nc.scalar.activation(
                out=S_sb[:],
                in_=S_sb[:],
                func=Exp,
                bias=nm32[:, 0:1],
                scale=float(ISCALE),
                accum_out=lsum[:],
            )
            nc.vector.reciprocal(linvall[:, qg : qg + 1], lsum[:])
            return S_sb

        def emit_transposes(j, qi, S_sb):
            for kc in range(KC):
                tp = tp_ps.tile([P, P], FR, tag="tp")
                nc.tensor.transpose(tp[:], S_sb[:, kc * P : (kc + 1) * P], ident[:])
                nc.scalar.copy(out=PT[:, kc, qi * P : (qi + 1) * P], in_=tp[:])

        def emit_b2_c(j):
            # B2: OT[dv, q] = sum_k v[k, dv] * PT[k, q]
            for g in range(2):
                pso = [
                    pv_ps.tile([P, QH], F32, name=f"pso{dl}") for dl in range(4)
                ]
                for kc in range(KC):
                    vt = vtp.tile([P, 512], FR, tag="vt", name=f"vt_{j}_{g}_{kc}")
                    nc.gpsimd.dma_start(
                        vt[:],
                        vspill[kc * P : (kc + 1) * P, g * 512 : (g + 1) * 512],
                    )
                    for dl in range(4):
                        nc.tensor.matmul(
                            pso[dl][:],
                            lhsT=vt[:, dl * P : (dl + 1) * P],
                            rhs=PT[:, kc, :],
                            start=(kc == 0),
                            stop=(kc == KC - 1),
                        )
                for dl in range(4):
                    nc.vector.tensor_copy(out=OT[:, g * 4 + dl, :], in_=pso[dl][:])

            # C: out[q, dout] = sum_dv OT[dv, q] * Wo[dv, dout]  (+ bo)
            for qi in range(4):
                outst = outp.tile([P, D], F32, tag="outst", name=f"outst_{j}_{qi}")
                for dh in range(2):
                    ps = mm_ps.tile([P, 512], F32, tag="mm")
                    for do in range(DK):
                        nc.tensor.matmul(
                            ps[:],
                            lhsT=OT[:, do, qi * P : (qi + 1) * P],
                            rhs=wsb4[:, do, dh * 512 : (dh + 1) * 512],
                            start=(do == 0),
                            stop=(do == DK - 1),
                        )
                    nc.vector.scalar_tensor_tensor(
                        out=outst[:, dh * 512 : (dh + 1) * 512],
                        in0=ps[:],
                        scalar=linvall[:, j * 4 + qi : j * 4 + qi + 1],
                        in1=borep[:, dh * 512 : (dh + 1) * 512],
                        op0=mybir.AluOpType.mult,
                        op1=mybir.AluOpType.add,
                    )
                row = j * QH + qi * P
                nc.sync.dma_start(out[row : row + P, :], outst[:])

        stages = [(j, qi) for j in range(NJOB) for qi in range(4)]
        pending = None  # (j, qi, S_sb) awaiting transposes
        for j, qi in stages:
            S_sb = emit_qk_softmax(j, qi)
            if pending is not None:
                pj, pqi, pS = pending
                emit_transposes(pj, pqi, pS)
                if pqi == 3:
                    emit_b2_c(pj)
            pending = (j, qi, S_sb)
        pj, pqi, pS = pending
        emit_transposes(pj, pqi, pS)
        emit_b2_c(pj)

    nc.compile()
    return nc


def _get_nc():
    if "nc" not in _CACHE:
        _CACHE["nc"] = _build_nc()
    return _CACHE["nc"]


def _make_masks():
    if "masks" not in _CACHE:
        import ml_dtypes

        masks = []
        kk = np.arange(S, dtype=np.int64)[None, :]
        for h in range(2):
            qr = (np.arange(QROWS, dtype=np.int64) + h * QROWS)[:, None]
            m = np.where(kk <= qr, np.float32(0.0), np.float32(NEG))
            masks.append(np.ascontiguousarray(m.astype(ml_dtypes.bfloat16)))
        _CACHE["masks"] = masks
    return _CACHE["masks"]


def kernel(query, key, value, Wq, bq, Wk, bk, Wv, bv, Wo, bo):
    from concourse.bass_utils import run_bass_kernel_spmd

    nc = _get_nc()
    masks = _make_masks()

    f32 = np.float32
    query = np.asarray(query, f32)
    key = np.asarray(key, f32)
    value = np.asarray(value, f32)
    Wq_, Wk_, Wv_, Wo_ = (np.ascontiguousarray(np.asarray(w, f32)) for w in (Wq, Wk, Wv, Wo))
    bqp = np.ascontiguousarray(np.asarray(bq, f32).reshape(DK, P).T)
    bkp = np.ascontiguousarray(np.asarray(bk, f32).reshape(DK, P).T)
    bvr = np.ascontiguousarray(np.asarray(bv, f32).reshape(1, D))
    bor = np.ascontiguousarray(np.asarray(bo, f32).reshape(1, D))

    in_maps = []
    xkT_b = [np.ascontiguousarray(key[b].T) for b in range(B)]
    xvT_b = [np.ascontiguousarray(value[b].T) for b in range(B)]
    for c in range(8):
        b, h = divmod(c, 2)
        in_maps.append(
            {
                "xqT": np.ascontiguousarray(query[b, h * QROWS : (h + 1) * QROWS, :].T),
                "xkT": xkT_b[b],
                "xvT": xvT_b[b],
                "Wq": Wq_,
                "Wk": Wk_,
                "Wv": Wv_,
                "Wo": Wo_,
                "bqp": bqp,
                "bkp": bkp,
                "bvr": bvr,
                "bor": bor,
                "mask": masks[h],
            }
        )

    res = None
    last_err = None
    for attempt in range(3):
        try:
            res = run_bass_kernel_spmd(nc, in_maps, core_ids=list(range(8)))
            break
        except Exception as e:  # transient NRT device errors: retry
            last_err = e
            import time as _time

            _time.sleep(5.0 * (attempt + 1))
    if res is None:
        raise last_err

    full = np.empty((B, S, D), dtype=f32)
    for c in range(8):
        b, h = divmod(c, 2)
        full[b, h * QROWS : (h + 1) * QROWS, :] = res.results[c]["out"]
    return full


# revision 31
# speedup vs baseline: 1.0107x; 1.0107x over previous
"""Trainium2 Bass kernel for single-head causal attention (B=4, S=2048, D=1024).

Sharding: 8 cores = 4 batches x 2 query-halves (1024 q-rows each). Every core
runs the identical program on full-length keys (2048); causality enters via a
per-core additive mask input, so one SPMD NEFF serves all cores.

Per-core dataflow (all f32 storage, float32r matmuls, fp32 PSUM accumulate):
  A1: kT[do,k]  = Wk^T @ xkT      (kT resident in SBUF, 64KB/part)
  A2: v[s,dv]   = xv @ Wv         (spilled to DRAM, re-streamed in B2)
  A3: qT[do,q]  = Wq^T @ xqT      (resident, 32KB/part)
  B1: S[q,k]    = qT^T kT + mask; softmax rows via 2x ACT-exp passes
      PT        = P^T per 128x128 PE transpose
  B2: OT[dv,q]  = sum_k v[k,dv] * PT[k,q]   (PSUM accum over k-chunks)
  C : out[q,d]  = OT^T @ Wo + bo
"""

import os
import sys

import numpy as np

for _p in ("/opt/trn_rl_repo",):
    if os.path.isdir(_p) and _p not in sys.path:
        sys.path.insert(0, _p)

B, S, D = 4, 2048, 1024
P = 128
DK = D // P          # 8 contraction chunks of 128
QROWS = S // 2       # 1024 q rows per core
NJOB = 2             # jobs per core (512 q rows each)
QH = QROWS // NJOB   # 512
KB = S // 512        # 4 key blocks of 512
KC = S // P          # 16 key chunks of 128
NEG = -1e9
ISCALE = 1.0 / np.sqrt(np.float32(D))

_CACHE = {}


def _build_nc():
    from contextlib import ExitStack

    import concourse.bass as bass
    import concourse.mybir as mybir
    import concourse.tile as tile
    from concourse import bacc
    from concourse.masks import make_identity

    F32 = mybir.dt.float32
    FR = mybir.dt.float32r
    Ident = mybir.ActivationFunctionType.Identity
    Exp = mybir.ActivationFunctionType.Exp
    Ln = mybir.ActivationFunctionType.Ln
    AX = mybir.AxisListType.X
    MAX = mybir.AluOpType.max

    nc = bacc.Bacc(None, target_bir_lowering=False, debug=False)

    xqT = nc.dram_tensor("xqT", [D, QROWS], FR, kind="ExternalInput")
    xkT = nc.dram_tensor("xkT", [D, S], FR, kind="ExternalInput")
    xvT = nc.dram_tensor("xvT", [D, S], FR, kind="ExternalInput")
    Wq = nc.dram_tensor("Wq", [D, D], FR, kind="ExternalInput")
    Wk = nc.dram_tensor("Wk", [D, D], FR, kind="ExternalInput")
    Wv = nc.dram_tensor("Wv", [D, D], FR, kind="ExternalInput")
    Wo = nc.dram_tensor("Wo", [D, D], FR, kind="ExternalInput")
    bqp = nc.dram_tensor("bqp", [P, DK], F32, kind="ExternalInput")
    bkp = nc.dram_tensor("bkp", [P, DK], F32, kind="ExternalInput")
    bvr = nc.dram_tensor("bvr", [1, D], F32, kind="ExternalInput")
    bor = nc.dram_tensor("bor", [1, D], F32, kind="ExternalInput")
    mask = nc.dram_tensor("mask", [QROWS, S], mybir.dt.bfloat16, kind="ExternalInput")
    out = nc.dram_tensor("out", [QROWS, D], F32, kind="ExternalOutput")
    vspill = nc.dram_tensor("vspill", [S, D], FR, kind="Internal")

    wq_r = Wq.rearrange("(po pi) n -> pi po n", pi=P)
    wk_r = Wk.rearrange("(po pi) n -> pi po n", pi=P)
    wv_r = Wv.rearrange("(po pi) n -> pi po n", pi=P)
    wo_r = Wo.rearrange("(po pi) n -> pi po n", pi=P)
    xq_r = xqT.rearrange("(po pi) n -> pi po n", pi=P)
    xk_r = xkT.rearrange("(po pi) n -> pi po n", pi=P)
    xv_r = xvT.rearrange("(po pi) n -> pi po n", pi=P)

    with ExitStack() as ctx:
        tc = ctx.enter_context(tile.TileContext(nc))

        const = ctx.enter_context(tc.tile_pool(name="const", bufs=1))
        wp = ctx.enter_context(tc.tile_pool(name="wp", bufs=1))
        ktp = ctx.enter_context(tc.tile_pool(name="ktp", bufs=1))
        qtp = ctx.enter_context(tc.tile_pool(name="qtp", bufs=1))

        mm_ps = ctx.enter_context(tc.tile_pool(name="mm_ps", bufs=2, space="PSUM"))
        tp_ps = ctx.enter_context(tc.tile_pool(name="tp_ps", bufs=2, space="PSUM"))
        pv_ps = ctx.enter_context(tc.tile_pool(name="pv_ps", bufs=1, space="PSUM"))

        # ---- constants (persistent) ----
        ident = const.tile([P, P], FR)
        bqp_sb = const.tile([P, DK], F32)
        bkp_sb = const.tile([P, DK], F32)
        bvrep = const.tile([P, D], F32, tag="brep")
        linvall = const.tile([P, DK], F32)

        # ---- persistent activations ----
        kT = ktp.tile([P, DK, S], FR)       # 64KB/part
        qT = qtp.tile([P, DK, QROWS], FR)   # 32KB/part

        wsb = wp.tile([P, DK, D], FR, tag="W")

        # ---- Phase A: projections ----
        with tc.tile_pool(name="astream", bufs=2) as ast:
            # A1: kT[do, k] = sum_di Wk[di,do] * xkT[di,k]  (+ bk)
            # issue order matters: DMA issue serializes ~1us each on Sync
            xkts = []
            xkt0 = ast.tile([P, DK, 512], FR, tag="xs", name="xkt0")
            nc.sync.dma_start(xkt0[:], xk_r[:, :, 0:512])
            xkts.append(xkt0)
            nc.sync.dma_start(wsb[:, :, 0:P], wk_r[:, :, 0:P])
            xkt1 = ast.tile([P, DK, 512], FR, tag="xs", name="xkt1")
            nc.sync.dma_start(xkt1[:], xk_r[:, :, 512:1024])
            xkts.append(xkt1)
            nc.sync.dma_start(wsb[:, :, P:512], wk_r[:, :, P:512])
            nc.sync.dma_start(wsb[:, :, 512:D], wk_r[:, :, 512:D])
            for kb in (2, 3):
                xkt = ast.tile([P, DK, 512], FR, tag="xs", name=f"xkt{kb}")
                nc.sync.dma_start(xkt[:], xk_r[:, :, kb * 512 : (kb + 1) * 512])
                xkts.append(xkt)
            # tiny constants go on the gpsimd queue (not behind 16MB of W/x)
            nc.gpsimd.dma_start(bkp_sb[:], bkp[:, :])
            nc.gpsimd.dma_start(bqp_sb[:], bqp[:, :])
            nc.gpsimd.dma_start(bvrep[:], bvr[0, :].partition_broadcast(P))
            ident32 = ast.tile([P, P], F32, tag="id32")
            make_identity(nc, ident32)
            nc.vector.tensor_copy(out=ident[:], in_=ident32[:])
            for kb in range(KB):
                xkt = xkts[kb]
                for do in range(DK):
                    gpool, gtag = ((mm_ps, "mm"), (tp_ps, "tp"))[(kb * DK + do) % 2]
                    ps = gpool.tile([P, 512], F32, tag=gtag)
                    for di in range(DK):
                        nc.tensor.matmul(
                            ps[:],
                            lhsT=wsb[:, di, do * P : (do + 1) * P],
                            rhs=xkt[:, di, :],
                            start=(di == 0),
                            stop=(di == DK - 1),
                        )
                    nc.scalar.activation(
                        out=kT[:, do, kb * 512 : (kb + 1) * 512],
                        in_=ps[:],
                        func=Ident,
                        bias=bkp_sb[:, do : do + 1],
                        scale=1.0,
                    )

            # A2: v[s, dv] = sum_di xvT[di,s] * Wv[di,dv]  (+ bv) -> spill DRAM
            wsb2 = wp.tile([P, DK, D], FR, tag="W")
            for wc in range(DK):
                nc.sync.dma_start(
                    wsb2[:, :, wc * P : (wc + 1) * P],
                    wv_r[:, :, wc * P : (wc + 1) * P],
                )
            for sb in range(4):
                xvt = ast.tile([P, DK, 512], FR, tag="xs", name=f"xvt{sb}")
                nc.sync.dma_start(xvt[:], xv_r[:, :, sb * 512 : (sb + 1) * 512])
                for sl in range(4):
                    sc = sb * 4 + sl
                    vstage = ast.tile([P, D], FR, tag="vstage", name=f"vst{sc}")
                    for dh in range(2):
                        gpool, gtag = ((mm_ps, "mm"), (tp_ps, "tp"))[(sc * 2 + dh) % 2]
                        ps = gpool.tile([P, 512], F32, tag=gtag)
                        for di in range(DK):
                            nc.tensor.matmul(
                                ps[:],
                                lhsT=xvt[:, di, sl * P : (sl + 1) * P],
                                rhs=wsb2[:, di, dh * 512 : (dh + 1) * 512],
                                start=(di == 0),
                                stop=(di == DK - 1),
                            )
                        nc.vector.tensor_add(
                            out=vstage[:, dh * 512 : (dh + 1) * 512],
                            in0=ps[:],
                            in1=bvrep[:, dh * 512 : (dh + 1) * 512],
                        )
                    nc.sync.dma_start(vspill[sc * P : (sc + 1) * P, :], vstage[:])

            # A3: qT[do, q] = sum_di Wq[di,do] * xqT[di,q]  (+ bq)
            wsb3 = wp.tile([P, DK, D], FR, tag="W")
            for wc in range(DK):
                nc.sync.dma_start(
                    wsb3[:, :, wc * P : (wc + 1) * P],
                    wq_r[:, :, wc * P : (wc + 1) * P],
                )
            for j in range(NJOB):
                xqt = ast.tile([P, DK, 512], FR, tag="xs")
                nc.sync.dma_start(xqt[:], xq_r[:, :, j * QH : (j + 1) * QH])
                for do in range(DK):
                    gpool, gtag = ((mm_ps, "mm"), (tp_ps, "tp"))[(j * DK + do) % 2]
                    ps = gpool.tile([P, 512], F32, tag=gtag)
                    for di in range(DK):
                        nc.tensor.matmul(
                            ps[:],
                            lhsT=wsb3[:, di, do * P : (do + 1) * P],
                            rhs=xqt[:, di, :],
                            start=(di == 0),
                            stop=(di == DK - 1),
                        )
                    nc.scalar.activation(
                        out=qT[:, do, j * QH : (j + 1) * QH],
                        in_=ps[:],
                        func=Ident,
                        bias=bqp_sb[:, do : do + 1],
                        scale=1.0,
                    )

        # Wo for phase C (single W slot; load ordered after Wq's last use)
        wsb4 = wp.tile([P, DK, D], FR, tag="W")
        for wc in range(DK):
            nc.sync.dma_start(
                wsb4[:, :, wc * P : (wc + 1) * P],
                wo_r[:, :, wc * P : (wc + 1) * P],
            )
        borep = const.tile([P, D], F32, tag="brep")
        nc.sync.dma_start(borep[:], bor[0, :].partition_broadcast(P))

        # ---- B/C-phase pools (allocated in the space astream released) ----
        ptp = ctx.enter_context(tc.tile_pool(name="ptp", bufs=1))
        otp = ctx.enter_context(tc.tile_pool(name="otp", bufs=1))
        sp = ctx.enter_context(tc.tile_pool(name="sp", bufs=2))
        mp = ctx.enter_context(tc.tile_pool(name="mp", bufs=1))
        vtp = ctx.enter_context(tc.tile_pool(name="vtp", bufs=3))
        outp = ctx.enter_context(tc.tile_pool(name="outp", bufs=1))
        statp = ctx.enter_context(tc.tile_pool(name="statp", bufs=1))

        PT = ptp.tile([P, KC, QH], FR)      # 32KB/part (per job, reused)
        OT = otp.tile([P, DK, QH], FR)      # 16KB/part (per job, reused)

        # ---- Phases B/C, software-pipelined across (job, q-subtile) ----
        # PE queue order: QK(s0) QK(s1) tp(s0) QK(s2) tp(s1) ... so each
        # stage's transposes wait out its softmax under the NEXT stage's QK
        # matmuls, keeping the PE dense (HAM stays at full clock).

        def emit_qk_softmax(j, qi):
            qg = j * 4 + qi
            S_sb = sp.tile([P, S], FR, tag="S", name=f"S_{qg}")
            mx4 = statp.tile([P, KB], F32, tag="mx4", name=f"mx4_{qg}")
            for kb in range(KB):
                mkt = mp.tile([P, 512], mybir.dt.bfloat16, tag="mask", name=f"mkt_{qg}_{kb}")
                nc.sync.dma_start(
                    mkt[:],
                    mask[qg * P : (qg + 1) * P, kb * 512 : (kb + 1) * 512],
                )
                ps = mm_ps.tile([P, 512], F32, tag="mm")
                for di in range(DK):
                    nc.tensor.matmul(
                        ps[:],
                        lhsT=qT[:, di, qg * P : (qg + 1) * P],
                        rhs=kT[:, di, kb * 512 : (kb + 1) * 512],
                        start=(di == 0),
                        stop=(di == DK - 1),
                    )
                nc.vector.tensor_add(
                    out=S_sb[:, kb * 512 : (kb + 1) * 512],
                    in0=ps[:],
                    in1=mkt[:],
                )
                nc.vector.tensor_reduce(
                    out=mx4[:, kb : kb + 1],
                    in_=S_sb[:, kb * 512 : (kb + 1) * 512],
                    axis=AX,
                    op=MAX,
                )
            mraw = statp.tile([P, 1], F32, tag="mraw", name=f"mraw_{qg}")
            nc.vector.tensor_reduce(out=mraw[:], in_=mx4[:], axis=AX, op=MAX)
            nm32 = statp.tile([P, 1], F32, tag="nm32", name=f"nm32_{qg}")
            nc.vector.tensor_scalar_mul(nm32[:], mraw[:], -float(ISCALE))
            # single in-place exp with fused row-sum, then scale by 1/l
            lsum = statp.tile([P, 1], F32, tag="lsum", name=f"lsum_{qg}")
            nc.scalar.activation(
                out=S_sb[:],
                in_=S_sb[:],
                func=Exp,
                bias=nm32[:, 0:1],
                scale=float(ISCALE),
                accum_out=lsum[:],
            )
            nc.vector.reciprocal(linvall[:, qg : qg + 1], lsum[:])
            return S_sb

        def emit_transposes(j, qi, S_sb):
            for kc in range(KC):
                tp = tp_ps.tile([P, P], FR, tag="tp")
                nc.tensor.transpose(tp[:], S_sb[:, kc * P : (kc + 1) * P], ident[:])
                if kc % 2 == 0:
                    nc.scalar.copy(out=PT[:, kc, qi * P : (qi + 1) * P], in_=tp[:])
                else:
                    nc.vector.tensor_copy(
                        out=PT[:, kc, qi * P : (qi + 1) * P], in_=tp[:]
                    )

        def emit_b2_c(j):
            # B2: OT[dv, q] = sum_k v[k, dv] * PT[k, q]
            for g in range(2):
                pso = [
                    pv_ps.tile([P, QH], F32, name=f"pso{dl}") for dl in range(4)
                ]
                for kc in range(KC):
                    vt = vtp.tile([P, 512], FR, tag="vt", name=f"vt_{j}_{g}_{kc}")
                    nc.gpsimd.dma_start(
                        vt[:],
                        vspill[kc * P : (kc + 1) * P, g * 512 : (g + 1) * 512],
                    )
                    for dl in range(4):
                        nc.tensor.matmul(
                            pso[dl][:],
                            lhsT=vt[:, dl * P : (dl + 1) * P],
                            rhs=PT[:, kc, :],
                            start=(kc == 0),
                            stop=(kc == KC - 1),
                        )
                for dl in range(4):
                    nc.vector.tensor_copy(out=OT[:, g * 4 + dl, :], in_=pso[dl][:])

            # C: out[q, dout] = sum_dv OT[dv, q] * Wo[dv, dout]  (+ bo)
            for qi in range(4):
                outst = outp.tile([P, D], F32, tag="outst", name=f"outst_{j}_{qi}")
                for dh in range(2):
                    ps = mm_ps.tile([P, 512], F32, tag="mm")
                    for do in range(DK):
                        nc.tensor.matmul(
                            ps[:],
                            lhsT=OT[:, do, qi * P : (qi + 1) * P],
                            rhs=wsb4[:, do, dh * 512 : (dh + 1) * 512],
                            start=(do == 0),
                            stop=(do == DK - 1),
                        )
                    nc.vector.scalar_tensor_tensor(
                        out=outst[:, dh * 512 : (dh + 1) * 512],
                        in0=ps[:],
                        scalar=linvall[:, j * 4 + qi : j * 4 + qi + 1],
                        in1=borep[:, dh * 512 : (dh + 1) * 512],
                        op0=mybir.AluOpType.mult,
                        op1=mybir.AluOpType.add,
                    )
                row = j * QH + qi * P
                nc.sync.dma_start(out[row : row + P, :], outst[:])

        stages = [(j, qi) for j in range(NJOB) for qi in range(4)]
        pending = None  # (j, qi, S_sb) awaiting transposes
        for j, qi in stages:
            S_sb = emit_qk_softmax(j, qi)
            if pending is not None:
                pj, pqi, pS = pending
                emit_transposes(pj, pqi, pS)
                if pqi == 3:
                    emit_b2_c(pj)
            pending = (j, qi, S_sb)
        pj, pqi, pS = pending
        emit_transposes(pj, pqi, pS)
        emit_b2_c(pj)

    nc.compile()
    return nc


def _get_nc():
    if "nc" not in _CACHE:
        _CACHE["nc"] = _build_nc()
    return _CACHE["nc"]


def _make_masks():
    if "masks" not in _CACHE:
        import ml_dtypes

        masks = []
        kk = np.arange(S, dtype=np.int64)[None, :]
        for h in range(2):
            qr = (np.arange(QROWS, dtype=np.int64) + h * QROWS)[:, None]
            m = np.where(kk <= qr, np.float32(0.0), np.float32(NEG))
            masks.append(np.ascontiguousarray(m.astype(ml_dtypes.bfloat16)))
        _CACHE["masks"] = masks
    return _CACHE["masks"]


def kernel(query, key, value, Wq, bq, Wk, bk, Wv, bv, Wo, bo):
    from concourse.bass_utils import run_bass_kernel_spmd

    nc = _get_nc()
    masks = _make_masks()

    f32 = np.float32
    query = np.asarray(query, f32)
    key = np.asarray(key, f32)
    value = np.asarray(value, f32)
    Wq_, Wk_, Wv_, Wo_ = (np.ascontiguousarray(np.asarray(w, f32)) for w in (Wq, Wk, Wv, Wo))
    bqp = np.ascontiguousarray(np.asarray(bq, f32).reshape(DK, P).T)
    bkp = np.ascontiguousarray(np.asarray(bk, f32).reshape(DK, P).T)
    bvr = np.ascontiguousarray(np.asarray(bv, f32).reshape(1, D))
    bor = np.ascontiguousarray(np.asarray(bo, f32).reshape(1, D))

    in_maps = []
    xkT_b = [np.ascontiguousarray(key[b].T) for b in range(B)]
    xvT_b = [np.ascontiguousarray(value[b].T) for b in range(B)]
    for c in range(8):
        b, h = divmod(c, 2)
        in_maps.append(
            {
                "xqT": np.ascontiguousarray(query[b, h * QROWS : (h + 1) * QROWS, :].T),
                "xkT": xkT_b[b],
                "xvT": xvT_b[b],
                "Wq": Wq_,
                "Wk": Wk_,
                "Wv": Wv_,
                "Wo": Wo_,
                "bqp": bqp,
                "bkp": bkp,
                "bvr": bvr,
                "bor": bor,
                "mask": masks[h],
            }
        )

    res = None
    last_err = None
    for attempt in range(3):
        try:
            res = run_bass_kernel_spmd(nc, in_maps, core_ids=list(range(8)))
            break
        except Exception as e:  # transient NRT device errors: retry
            last_err = e
            import time as _time

            _time.sleep(5.0 * (attempt + 1))
    if res is None:
        raise last_err

    full = np.empty((B, S, D), dtype=f32)
    for c in range(8):
        b, h = divmod(c, 2)
        full[b, h * QROWS : (h + 1) * QROWS, :] = res.results[c]["out"]
    return full
